# revision 3
# baseline (speedup 1.0000x reference)
"""Trainium2 Bass kernel for nn_CanonicalCov1D (strided dual-projection covariance).

v3: bf16 baseline structure with
  - host-side im2col (winT built on host, contiguous DMA load; no xbar
    transpose, no DMA-mode serialization in the prologue)
  - role-swapped products: scalar exits the p2 singles to SBUF bf16, the
    DVE does ONE FD-2NT mixed mul per pair-chunk against the 2-bank
    p1pair PSUM tile (halves DVE op count + semaphore traffic)
  - selector matmul emitted one iteration late so the in-order PE queue
    never blocks projection matmuls on the DVE/gpsimd chain

Math: see reference. Weights centered on host (centering over LAT
commutes with the projection); win2[n] = win_full[n+16]; l-reduction as
a constant-selector matmul accumulating over chunk pairs in PSUM.
"""

import numpy as np

B, T = 32, 8192
W, LAT, C = 128, 32, 64
STRIDE = 8
NWIN = 993
NPAD = 1040
N_CORES = 8
BPC = B // N_CORES
NTILES = 2
NCHUNKS = 16

_CACHE = {}


def _build():
    import concourse.bass as bass
    import concourse.mybir as mybir
    import concourse.tile as tile
    from concourse import bacc

    f32 = mybir.dt.float32
    bf16 = mybir.dt.bfloat16

    nc = bacc.Bacc("TRN2", target_bir_lowering=False, debug=False,
                   enable_asserts=False)

    x_dram = nc.dram_tensor("x", [BPC, W, NPAD], bf16, kind="ExternalInput")
    w_dram = nc.dram_tensor("w", [W, 2 * LAT * C], bf16, kind="ExternalInput")
    sel_dram = nc.dram_tensor("sel", [W, C], bf16, kind="ExternalInput")
    bias_dram = nc.dram_tensor("bias", [C, 1], f32, kind="ExternalInput")
    out_dram = nc.dram_tensor("out", [BPC, C, NWIN], f32, kind="ExternalOutput")

    with tile.TileContext(nc) as tc:
        with (
            tc.tile_pool(name="consts", bufs=1) as consts,
            tc.tile_pool(name="wins", bufs=1) as wins,
            tc.tile_pool(name="prods", bufs=4) as prods,
            tc.tile_pool(name="outs", bufs=2) as outs,
            tc.tile_pool(name="psum", bufs=1, space="PSUM") as psum,
        ):
            # prologue DMAs, first-needed first (all plain copies, no xbar)
            w_sb = consts.tile([W, 2 * LAT * C], bf16)
            nc.sync.dma_start(w_sb[:, 0:1024], w_dram.ap()[:, 0:1024])
            winTs = []
            for b in range(BPC):
                wt = wins.tile([W, NPAD], bf16, name=f"winT{b}", tag="winT",
                               bufs=4)
                nc.sync.dma_start(wt[:], x_dram.ap()[b])
                winTs.append(wt)
                if b == 0:
                    nc.sync.dma_start(w_sb[:, 2048:3072],
                                      w_dram.ap()[:, 2048:3072])
            nc.sync.dma_start(w_sb[:, 1024:2048], w_dram.ap()[:, 1024:2048])
            nc.sync.dma_start(w_sb[:, 3072:4096], w_dram.ap()[:, 3072:4096])
            sel_sb = consts.tile([W, C], bf16)
            nc.sync.dma_start(sel_sb[:], sel_dram.ap())
            bias_sb = consts.tile([C, 1], f32)
            nc.sync.dma_start(bias_sb[:], bias_dram.ap())

            # warmup matmuls: ramp the PE clock gate during the DMA prologue
            warm_sb = consts.tile([128, 64], bf16)
            nc.gpsimd.memset(warm_sb[:], 0.0)
            warm_ps = psum.tile([64, 512], f32, tag="selout", bufs=1)
            for i in range(20):
                nc.tensor.matmul(
                    warm_ps[:],
                    warm_sb[:],
                    warm_sb[:, None, :].to_broadcast((128, 8, 64)),
                    start=(i == 0),
                    stop=(i == 19),
                )

            for b in range(BPC):
                winT = winTs[b]
                for t in range(NTILES):
                    NT = 512 if t == 0 else NWIN - 512
                    n0 = t * 512
                    selout = psum.tile([C, 512], f32, tag="selout", bufs=1)
                    pend = None  # deferred selector input (previous p12sum)
                    for jp in range(NCHUNKS // 2):
                        ca, cb = 2 * jp, 2 * jp + 1
                        # p2 pair first: 2-bank tile consumed by the DVE mul
                        p2pair = psum.tile([128, 1024], f32, tag="p2pair",
                                           bufs=2)
                        for qi, j in enumerate((ca, cb)):
                            nc.tensor.matmul(
                                p2pair[:, qi * 512 : qi * 512 + NT],
                                w_sb[:, 2048 + j * 128 : 2048 + j * 128 + 128],
                                winT[:, n0 + 16 : n0 + 16 + NT],
                                start=True, stop=True,
                            )

                        # selector for the PREVIOUS pair-chunk: emitted mid-
                        # iteration so the in-order PE queue never stalls
                        # projections on the DVE/gpsimd chain
                        if pend is not None:
                            nc.tensor.matmul(
                                selout[:, 0:NT], sel_sb[:], pend[:, 0:NT],
                                start=(jp == 1),
                                stop=False,
                            )

                        # p1 pair: consumed by the scalar copy (bufs=1)
                        p1pair = psum.tile([128, 1024], f32, tag="p1pair",
                                           bufs=1)
                        for qi, j in enumerate((ca, cb)):
                            nc.tensor.matmul(
                                p1pair[:, qi * 512 : qi * 512 + NT],
                                w_sb[:, j * 128 : j * 128 + 128],
                                winT[:, n0 : n0 + NT],
                                start=True, stop=True,
                            )

                        # scalar: one strided exit of the p1 pair to bf16
                        p1c = prods.tile([128, 1024], bf16, tag="p1c", bufs=4)
                        nc.scalar.copy(
                            p1c.rearrange("p (q n) -> p q n", q=2)[:, :, 0:NT],
                            p1pair.rearrange("p (q n) -> p q n", q=2)[:, :, 0:NT],
                        )
                        # DVE: one FD-2NT mixed mul vs the 2-bank p2pair
                        p12pair = prods.tile([128, 1024], bf16, tag="p12",
                                             bufs=4)
                        nc.vector.tensor_mul(
                            p12pair.rearrange("p (q n) -> p q n", q=2)[:, :, 0:NT],
                            p1c.rearrange("p (q n) -> p q n", q=2)[:, :, 0:NT],
                            p2pair.rearrange("p (q n) -> p q n", q=2)[:, :, 0:NT],
                        )
                        # gpsimd: pre-add the chunk pair, halving selector mms
                        p12sum = prods.tile([128, 512], bf16, tag="p12sum",
                                            bufs=4)
                        nc.gpsimd.tensor_add(
                            p12sum[:, 0:NT],
                            p12pair[:, 0:NT],
                            p12pair[:, 512 : 512 + NT],
                        )
                        pend = p12sum

                    nc.tensor.matmul(
                        selout[:, 0:NT], sel_sb[:], pend[:, 0:NT],
                        start=False, stop=True,
                    )
                    st = outs.tile([C, 512], f32, tag="st")
                    nc.scalar.activation(
                        st[:, 0:NT],
                        selout[:, 0:NT],
                        mybir.ActivationFunctionType.Identity,
                        bias=bias_sb[:],
                    )
                    nc.sync.dma_start(
                        out_dram.ap()[b, :, n0 : n0 + NT], st[:, 0:NT]
                    )

    nc.compile()
    return nc


def _prep_inputs(X, weight1, weight2, bias):
    import ml_dtypes

    bf = ml_dtypes.bfloat16
    X = np.asarray(X, dtype=np.float32)
    weight1 = np.asarray(weight1, dtype=np.float32)
    weight2 = np.asarray(weight2, dtype=np.float32)
    bias = np.asarray(bias, dtype=np.float32)

    w1c = weight1 - weight1.mean(axis=1, keepdims=True)
    w2c = weight2 - weight2.mean(axis=1, keepdims=True)
    w1p = (w1c / LAT).reshape(W, LAT * C)
    w2p = w2c.reshape(W, LAT * C)
    wcat = np.concatenate([w1p, w2p], axis=1).astype(bf)

    xpad = np.zeros((B, T + 256), dtype=np.float32)
    xpad[:, :T] = X
    xb = np.ascontiguousarray(xpad.astype(bf))

    # host im2col: winT[b, w, n] = X[b, 8n + w]
    s = xb.strides
    wt = np.lib.stride_tricks.as_strided(
        xb, shape=(B, W, NPAD), strides=(s[0], s[1], s[1] * 8)
    )

    sel = (np.arange(W)[:, None] % C == np.arange(C)[None, :]).astype(bf)
    bias_col = np.ascontiguousarray(bias[:, None]).astype(np.float32)

    in_maps = []
    for i in range(N_CORES):
        in_maps.append({
            "x": np.ascontiguousarray(wt[i * BPC : (i + 1) * BPC]),
            "w": wcat,
            "sel": sel,
            "bias": bias_col,
        })
    return in_maps


def run_with_results(X, weight1, weight2, bias, trace=False, trace_cores=None):
    from concourse import bass_utils

    if "nc" not in _CACHE:
        _CACHE["nc"] = _build()
    nc = _CACHE["nc"]
    in_maps = _prep_inputs(X, weight1, weight2, bias)
    res = bass_utils.run_bass_kernel_spmd(
        nc, in_maps, core_ids=list(range(N_CORES)),
        trace=trace, trace_cores=trace_cores,
    )
    out = np.concatenate(
        [res.results[i]["out"] for i in range(N_CORES)], axis=0
    ).transpose(0, 2, 1)
    return np.ascontiguousarray(out, dtype=np.float32), res


def kernel(**inputs):
    out, _ = run_with_results(
        inputs["X"], inputs["weight1"], inputs["weight2"], inputs["bias"]
    )
    return out


# revision 4
# speedup vs baseline: 1.3497x; 1.3497x over previous
"""Trainium2 Bass kernel for nn_CanonicalCov1D (strided dual-projection covariance).

v3: bf16 baseline structure with
  - host-side im2col (winT built on host, contiguous DMA load; no xbar
    transpose, no DMA-mode serialization in the prologue)
  - role-swapped products: scalar exits the p2 singles to SBUF bf16, the
    DVE does ONE FD-2NT mixed mul per pair-chunk against the 2-bank
    p1pair PSUM tile (halves DVE op count + semaphore traffic)
  - selector matmul emitted one iteration late so the in-order PE queue
    never blocks projection matmuls on the DVE/gpsimd chain

Math: see reference. Weights centered on host (centering over LAT
commutes with the projection); win2[n] = win_full[n+16]; l-reduction as
a constant-selector matmul accumulating over chunk pairs in PSUM.
"""

import numpy as np

B, T = 32, 8192
W, LAT, C = 128, 32, 64
STRIDE = 8
NWIN = 993
NPAD = 1040
N_CORES = 8
BPC = B // N_CORES
NTILES = 2
NCHUNKS = 16

_CACHE = {}


def _build():
    import concourse.bass as bass
    import concourse.mybir as mybir
    import concourse.tile as tile
    from concourse import bacc

    f32 = mybir.dt.float32
    bf16 = mybir.dt.bfloat16

    nc = bacc.Bacc("TRN2", target_bir_lowering=False, debug=False,
                   enable_asserts=False)

    x_dram = nc.dram_tensor("x", [BPC, W, NPAD], bf16, kind="ExternalInput")
    w_dram = nc.dram_tensor("w", [W, 2 * LAT * C], bf16, kind="ExternalInput")
    sel_dram = nc.dram_tensor("sel", [W, C], bf16, kind="ExternalInput")
    bias_dram = nc.dram_tensor("bias", [C, 1], f32, kind="ExternalInput")
    out_dram = nc.dram_tensor("out", [BPC, C, NWIN], f32, kind="ExternalOutput")

    with tile.TileContext(nc) as tc:
        with (
            tc.tile_pool(name="consts", bufs=1) as consts,
            tc.tile_pool(name="wins", bufs=1) as wins,
            tc.tile_pool(name="prods", bufs=4) as prods,
            tc.tile_pool(name="outs", bufs=2) as outs,
            tc.tile_pool(name="psum", bufs=1, space="PSUM") as psum,
        ):
            # prologue DMAs, first-needed first (all plain copies, no xbar)
            w_sb = consts.tile([W, 2 * LAT * C], bf16)
            nc.sync.dma_start(w_sb[:, 0:1024], w_dram.ap()[:, 0:1024])
            winTs = []
            for b in range(BPC):
                wt = wins.tile([W, NPAD], bf16, name=f"winT{b}", tag="winT",
                               bufs=4)
                nc.sync.dma_start(wt[:], x_dram.ap()[b])
                winTs.append(wt)
                if b == 0:
                    nc.sync.dma_start(w_sb[:, 2048:3072],
                                      w_dram.ap()[:, 2048:3072])
            nc.sync.dma_start(w_sb[:, 1024:2048], w_dram.ap()[:, 1024:2048])
            nc.sync.dma_start(w_sb[:, 3072:4096], w_dram.ap()[:, 3072:4096])
            sel_sb = consts.tile([W, C], bf16)
            nc.sync.dma_start(sel_sb[:], sel_dram.ap())
            bias_sb = consts.tile([C, 1], f32)
            nc.sync.dma_start(bias_sb[:], bias_dram.ap())

            # warmup matmuls: ramp the PE clock gate during the DMA prologue
            warm_sb = consts.tile([128, 64], bf16)
            nc.gpsimd.memset(warm_sb[:], 0.0)
            warm_ps = psum.tile([64, 512], f32, tag="selout", bufs=1)
            for i in range(20):
                nc.tensor.matmul(
                    warm_ps[:],
                    warm_sb[:],
                    warm_sb[:, None, :].to_broadcast((128, 8, 64)),
                    start=(i == 0),
                    stop=(i == 19),
                )

            for b in range(BPC):
                winT = winTs[b]
                for t in range(NTILES):
                    NT = 512 if t == 0 else NWIN - 512
                    n0 = t * 512
                    selout = psum.tile([C, 512], f32, tag="selout", bufs=1)
                    pend = None  # deferred selector input (previous p12sum)
                    for jp in range(NCHUNKS // 2):
                        ca, cb = 2 * jp, 2 * jp + 1
                        # alternate scalar-heavy / DVE-heavy product paths to
                        # balance the two engines (both ~100% otherwise)
                        v1_style = (jp % 2 == 0)
                        p1pair = psum.tile([128, 1024], f32, tag="p1pair",
                                           bufs=2)
                        p2s = []
                        for qi, j in enumerate((ca, cb)):
                            nc.tensor.matmul(
                                p1pair[:, qi * 512 : qi * 512 + NT],
                                w_sb[:, j * 128 : j * 128 + 128],
                                winT[:, n0 : n0 + NT],
                                start=True, stop=True,
                            )
                            p2 = psum.tile([128, 512], f32, tag="p2", bufs=3)
                            nc.tensor.matmul(
                                p2[:, 0:NT],
                                w_sb[:, 2048 + j * 128 : 2048 + j * 128 + 128],
                                winT[:, n0 + 16 : n0 + 16 + NT],
                                start=True, stop=True,
                            )
                            p2s.append(p2)

                        # selector for the PREVIOUS pair-chunk: emitted mid-
                        # iteration so the in-order PE queue never stalls
                        # projections on the DVE/gpsimd chain
                        if pend is not None:
                            nc.tensor.matmul(
                                selout[:, 0:NT], sel_sb[:], pend[:, 0:NT],
                                start=(jp == 1),
                                stop=False,
                            )

                        if v1_style:
                            # scalar: one strided FD-2NT exit of the p1 pair;
                            # DVE: two FD-NT mixed muls vs the p2 singles
                            p1c = prods.tile([128, 1024], bf16, tag="p1c",
                                             bufs=4)
                            nc.scalar.copy(
                                p1c.rearrange("p (q n) -> p q n", q=2)[:, :, 0:NT],
                                p1pair.rearrange("p (q n) -> p q n", q=2)[:, :, 0:NT],
                            )
                            p12a = prods.tile([128, 512], bf16, tag="p12",
                                              bufs=8)
                            p12b = prods.tile([128, 512], bf16, tag="p12",
                                              bufs=8)
                            for qi, p12 in enumerate((p12a, p12b)):
                                nc.vector.tensor_mul(
                                    p12[:, 0:NT],
                                    p1c[:, qi * 512 : qi * 512 + NT],
                                    p2s[qi][:, 0:NT],
                                )
                            adds = (p12a[:, 0:NT], p12b[:, 0:NT])
                        else:
                            # scalar: two FD-NT exits of the p2 singles;
                            # DVE: one FD-2NT mixed mul vs the 2-bank p1pair
                            p2c = prods.tile([128, 1024], bf16, tag="p2c",
                                             bufs=4)
                            for qi in range(2):
                                nc.scalar.copy(
                                    p2c[:, qi * 512 : qi * 512 + NT],
                                    p2s[qi][:, 0:NT],
                                )
                            p12pair = prods.tile([128, 1024], bf16, tag="p12",
                                                 bufs=8)
                            nc.vector.tensor_mul(
                                p12pair.rearrange("p (q n) -> p q n", q=2)[:, :, 0:NT],
                                p2c.rearrange("p (q n) -> p q n", q=2)[:, :, 0:NT],
                                p1pair.rearrange("p (q n) -> p q n", q=2)[:, :, 0:NT],
                            )
                            adds = (p12pair[:, 0:NT],
                                    p12pair[:, 512 : 512 + NT])

                        # gpsimd: pre-add the chunk pair, halving selector mms
                        p12sum = prods.tile([128, 512], bf16, tag="p12sum",
                                            bufs=4)
                        nc.gpsimd.tensor_add(p12sum[:, 0:NT], *adds)
                        pend = p12sum

                    nc.tensor.matmul(
                        selout[:, 0:NT], sel_sb[:], pend[:, 0:NT],
                        start=False, stop=True,
                    )
                    st = outs.tile([C, 512], f32, tag="st")
                    nc.scalar.activation(
                        st[:, 0:NT],
                        selout[:, 0:NT],
                        mybir.ActivationFunctionType.Identity,
                        bias=bias_sb[:],
                    )
                    nc.sync.dma_start(
                        out_dram.ap()[b, :, n0 : n0 + NT], st[:, 0:NT]
                    )

    nc.compile()
    return nc


def _prep_inputs(X, weight1, weight2, bias):
    import ml_dtypes

    bf = ml_dtypes.bfloat16
    X = np.asarray(X, dtype=np.float32)
    weight1 = np.asarray(weight1, dtype=np.float32)
    weight2 = np.asarray(weight2, dtype=np.float32)
    bias = np.asarray(bias, dtype=np.float32)

    w1c = weight1 - weight1.mean(axis=1, keepdims=True)
    w2c = weight2 - weight2.mean(axis=1, keepdims=True)
    w1p = (w1c / LAT).reshape(W, LAT * C)
    w2p = w2c.reshape(W, LAT * C)
    wcat = np.concatenate([w1p, w2p], axis=1).astype(bf)

    xpad = np.zeros((B, T + 256), dtype=np.float32)
    xpad[:, :T] = X
    xb = np.ascontiguousarray(xpad.astype(bf))

    # host im2col: winT[b, w, n] = X[b, 8n + w]
    s = xb.strides
    wt = np.lib.stride_tricks.as_strided(
        xb, shape=(B, W, NPAD), strides=(s[0], s[1], s[1] * 8)
    )

    sel = (np.arange(W)[:, None] % C == np.arange(C)[None, :]).astype(bf)
    bias_col = np.ascontiguousarray(bias[:, None]).astype(np.float32)

    in_maps = []
    for i in range(N_CORES):
        in_maps.append({
            "x": np.ascontiguousarray(wt[i * BPC : (i + 1) * BPC]),
            "w": wcat,
            "sel": sel,
            "bias": bias_col,
        })
    return in_maps


def run_with_results(X, weight1, weight2, bias, trace=False, trace_cores=None):
    from concourse import bass_utils

    if "nc" not in _CACHE:
        _CACHE["nc"] = _build()
    nc = _CACHE["nc"]
    in_maps = _prep_inputs(X, weight1, weight2, bias)
    res = bass_utils.run_bass_kernel_spmd(
        nc, in_maps, core_ids=list(range(N_CORES)),
        trace=trace, trace_cores=trace_cores,
    )
    out = np.concatenate(
        [res.results[i]["out"] for i in range(N_CORES)], axis=0
    ).transpose(0, 2, 1)
    return np.ascontiguousarray(out, dtype=np.float32), res


def kernel(**inputs):
    out, _ = run_with_results(
        inputs["X"], inputs["weight1"], inputs["weight2"], inputs["bias"]
    )
    return out
